# revision 8
# baseline (speedup 1.0000x reference)
"""Trainium2 Bass kernel for gated-attention segment pooling (nn_Pooler).

Algorithm (per pooling kind — nodes and edges are independent):
    logits = MLP(x)             # 256 -> 128 -> 64 -> 1, relu between
    e      = exp(logits)        # b3 dropped: exp(b3) cancels in softmax
    out[g] = sum_i e_i * x_i / sum_i e_i   over segment g (indices sorted)

Sharding: 2048 segments -> 8 cores x 256 contiguous segments. Each core
processes its own row range; segments never straddle cores, so there is no
cross-device reduction. Within a core, segments are processed in NB=2 blocks
of SEGB=128 segments; each block's rows are padded to a whole number of
128-row chunks so the PSUM accumulator [128 segs, 257] has a fixed-size
matmul accumulation group.

Per 128-row chunk (all matmul inputs fp16, PSUM accumulation fp32):
    h1   [128,128] = W1h0.T @ XTh0 + W1h1.T @ XTh1     (PE)
    h1s  = relu(h1 + b1) -> fp16                        (ACT)
    h2   [64,128]  = W2.T @ h1s                         (PE)
    h2s  = relu(h2 + b2) -> fp16                        (DVE)
    lg   [128,1]   = h2s.T @ W3                         (PE)
    e    = exp(lg)                                      (ACT)
    A    [128,128] = (iota == segid) * e  -> fp16       (DVE)
    U   += A.T @ [X | 1]   [128 segs, 257]              (PE, PSUM accum)
Block end: out = U[:, :256] * reciprocal(U[:, 256] + eps).

The host pre-transposes X into XT (feature-major) so no on-device transpose
is needed; X and XT are both fp16 so total HBM traffic per row is ~1x fp32.
"""

import json
import math
from contextlib import ExitStack

import numpy as np

import concourse.bass as bass
import concourse.bass2jax as bass2jax
import concourse.bass_utils as bass_utils
import concourse.mybir as mybir
import concourse.tile as tile
from concourse.bass_utils import run_bass_kernel_spmd

# ---------------------------------------------------------------------------
# Sync-wait legalization.
#
# The walrus build in this container rejects instructions whose sync_info
# carries more than one wait ("Too many sync wait commands"; EventSemaphore
# instructions allow two — see bass_rust.inst_waits_full). Tile's scheduler
# emits multi-wait instructions, so before compiling we hoist excess waits
# into standalone EventSemaphore instructions on the same engine queue just
# before the instruction (queue order makes this semantically identical).
# ---------------------------------------------------------------------------

_WAIT_CAP = {"EventSemaphore": 2}
_orig_compile_bir_kernel = bass_utils.compile_bir_kernel


def _legalize_sync_waits(bir_json) -> bytes:
    j = json.loads(bir_json)
    n_new = 0
    for fn in j.get("functions", []):
        for blk in fn.get("blocks", []):
            insts = blk.get("instructions")
            if not insts:
                continue
            out = []
            changed = False
            for ins in insts:
                si = ins.get("sync_info")
                waits = (si or {}).get("on_wait") or []
                cap = _WAIT_CAP.get(ins.get("opcode"), 1)
                if len(waits) > cap:
                    hoist, keep = waits[:-cap], waits[-cap:]
                    for i in range(0, len(hoist), 2):
                        n_new += 1
                        ev = {
                            "engine": ins["engine"],
                            "ins": [],
                            "outs": [],
                            "name": f"syncw-{n_new}-{ins.get('name', '')}",
                            "opcode": "EventSemaphore",
                            "sync_info": {"on_update": [],
                                          "on_wait": hoist[i:i + 2]},
                        }
                        if "debug" in ins:
                            ev["debug"] = ins["debug"]
                        out.append(ev)
                    si["on_wait"] = keep
                    changed = True
                out.append(ins)
            if changed:
                blk["instructions"] = out
    return json.dumps(j).encode()


def _patched_compile_bir_kernel(bir_json, tmpdir, neff_name="file.neff"):
    return _orig_compile_bir_kernel(_legalize_sync_waits(bir_json), tmpdir,
                                    neff_name)


bass_utils.compile_bir_kernel = _patched_compile_bir_kernel
bass2jax.compile_bir_kernel = _patched_compile_bir_kernel

P = 128                      # rows per chunk / partition dim
SEGB = 128                   # segments per PSUM block
N_CORES = 8
N_SEG = 2048
XG = 8                       # chunks per DMA group (1 MiB-ish loads)
NB = (N_SEG // N_CORES) // SEGB   # blocks per core per kind
F16 = mybir.dt.float16
F32 = mybir.dt.float32
RELU = mybir.ActivationFunctionType.Relu
EXP = mybir.ActivationFunctionType.Exp
ADD = mybir.AluOpType.add
MAX = mybir.AluOpType.max
MULT = mybir.AluOpType.mult
ISEQ = mybir.AluOpType.is_equal


def _shard(x, idx, cb):
    """Pack one kind's rows into per-core padded arrays.

    Returns per-core lists: x_packed [GP, P, 257] f16, xt [2, P, R] f16,
    sg [P, G] f32."""
    segs_per_core = N_SEG // N_CORES
    bounds = np.searchsorted(idx, np.arange(0, N_SEG + 1, SEGB))
    g = NB * cb
    gp = int(math.ceil(g / XG)) * XG
    r = gp * P
    xs, xts, sgs = [], [], []
    for c in range(N_CORES):
        xp = np.zeros((r, 257), np.float16)
        xp[:, 256] = 1.0
        sg = np.full((g * P,), float(SEGB), np.float32)
        for b in range(NB):
            lo, hi = int(bounds[c * NB + b]), int(bounds[c * NB + b + 1])
            n = hi - lo
            dst = b * cb * P
            xp[dst:dst + n, :256] = x[lo:hi]
            sg[dst:dst + n] = (
                idx[lo:hi] - (c * segs_per_core + b * SEGB)
            ).astype(np.float32)
        xt = np.ascontiguousarray(xp[:, :256].T).reshape(2, P, r)
        xs.append(np.ascontiguousarray(xp.reshape(gp, P, 257)))
        xts.append(xt)
        sgs.append(np.ascontiguousarray(sg.reshape(g, P).T))
    return xs, xts, sgs


def _cb_for(idx):
    bounds = np.searchsorted(idx, np.arange(0, N_SEG + 1, SEGB))
    return int(np.ceil(np.diff(bounds).max() / P))


def _trace(ctx, tc, kinds, iota_d):
    nc = tc.nc
    const = ctx.enter_context(tc.tile_pool(name="const", bufs=1))
    xpool = ctx.enter_context(tc.tile_pool(name="xg", bufs=3))
    xtpool = ctx.enter_context(tc.tile_pool(name="xtg", bufs=3))
    spool = ctx.enter_context(tc.tile_pool(name="sm", bufs=3))
    opool = ctx.enter_context(tc.tile_pool(name="ob", bufs=2))
    ps_h1 = ctx.enter_context(tc.tile_pool(name="ph1", bufs=2, space="PSUM"))
    ps_h2 = ctx.enter_context(tc.tile_pool(name="ph2", bufs=2, space="PSUM"))
    ps_lg = ctx.enter_context(tc.tile_pool(name="plg", bufs=2, space="PSUM"))
    ps_u = ctx.enter_context(tc.tile_pool(name="pu", bufs=2, space="PSUM"))

    iota_t = const.tile([P, SEGB], F32, tag="iota")
    nc.sync.dma_start(iota_t[:], iota_d[:])

    for kd in kinds:
        tg = kd["tag"]
        cb, g = kd["cb"], kd["g"]
        w1_t = const.tile([P, 2 * P], F16, tag=f"w1{tg}")
        nc.sync.dma_start(w1_t[:], kd["w1"][:].rearrange("h p m -> p h m"))
        w2_t = const.tile([P, 64], F16, tag=f"w2{tg}")
        nc.sync.dma_start(w2_t[:], kd["w2"][:])
        w3_t = const.tile([64, 1], F16, tag=f"w3{tg}")
        nc.sync.dma_start(w3_t[:], kd["w3"][:])
        b1_t = const.tile([P, 1], F32, tag=f"b1{tg}")
        nc.sync.dma_start(b1_t[:], kd["b1"][:])
        b2_t = const.tile([64, 1], F32, tag=f"b2{tg}")
        nc.sync.dma_start(b2_t[:], kd["b2"][:])
        sg_t = const.tile([P, g], F32, tag=f"sg{tg}")
        nc.sync.dma_start(sg_t[:], kd["sg"][:])

        x_t = xt_t = None
        for b in range(NB):
            u_ps = ps_u.tile([SEGB, 257], F32, tag="u")
            for c in range(cb):
                gi = b * cb + c
                j = gi % XG
                if j == 0:
                    g8 = gi // XG
                    x_t = xpool.tile([P, XG * 257], F16, tag="xg")
                    nc.sync.dma_start(
                        x_t[:],
                        kd["x"][g8 * XG:(g8 + 1) * XG].rearrange("g p f -> p g f"),
                    )
                    xt_t = xtpool.tile([P, 2 * XG * P], F16, tag="xtg")
                    nc.sync.dma_start(
                        xt_t[:],
                        kd["xt"][:, :, g8 * XG * P:(g8 + 1) * XG * P].rearrange(
                            "h p r -> p h r"
                        ),
                    )
                h1_ps = ps_h1.tile([P, P], F32, tag="h1")
                for h in range(2):
                    nc.tensor.matmul(
                        h1_ps[:],
                        w1_t[:, h * P:(h + 1) * P],
                        xt_t[:, h * XG * P + j * P: h * XG * P + (j + 1) * P],
                        start=(h == 0),
                        stop=(h == 1),
                    )
                h1_sb = spool.tile([P, P], F16, tag="h1sb")
                nc.scalar.activation(h1_sb[:], h1_ps[:], RELU, bias=b1_t[:])
                h2_ps = ps_h2.tile([64, P], F32, tag="h2")
                nc.tensor.matmul(h2_ps[:], w2_t[:], h1_sb[:], start=True, stop=True)
                h2_sb = spool.tile([64, P], F16, tag="h2sb")
                nc.vector.tensor_scalar(
                    out=h2_sb[:], in0=h2_ps[:],
                    scalar1=b2_t[:], scalar2=0.0, op0=ADD, op1=MAX,
                )
                lg_ps = ps_lg.tile([P, 1], F32, tag="lg")
                nc.tensor.matmul(lg_ps[:], h2_sb[:], w3_t[:], start=True, stop=True)
                e_sb = spool.tile([P, 1], F32, tag="e")
                nc.scalar.activation(e_sb[:], lg_ps[:], EXP)
                a_sb = spool.tile([P, SEGB], F16, tag="a")
                nc.vector.tensor_scalar(
                    out=a_sb[:], in0=iota_t[:],
                    scalar1=sg_t[:, gi:gi + 1], scalar2=e_sb[:],
                    op0=ISEQ, op1=MULT,
                )
                nc.tensor.matmul(
                    u_ps[:], a_sb[:], x_t[:, j * 257:(j + 1) * 257],
                    start=(c == 0), stop=(c == cb - 1), skip_group_check=True,
                )
            sr = spool.tile([SEGB, 1], F32, tag="sr")
            nc.vector.tensor_scalar(
                out=sr[:], in0=u_ps[:, 256:257], scalar1=1e-30, scalar2=None,
                op0=ADD,
            )
            rc = spool.tile([SEGB, 1], F32, tag="rc")
            nc.vector.reciprocal(rc[:], sr[:])
            o_sb = opool.tile([SEGB, 256], F32, tag="o")
            nc.vector.tensor_scalar(
                out=o_sb[:], in0=u_ps[:, :256], scalar1=rc[:], scalar2=None,
                op0=MULT,
            )
            nc.sync.dma_start(kd["out"][b * SEGB:(b + 1) * SEGB, :], o_sb[:])


def _build(cb_n, cb_e):
    nc = bass.Bass(
        "TRN2", target_bir_lowering=False, debug=False, enable_asserts=False,
        num_devices=N_CORES,
    )
    kinds = []
    for tg, cb in (("n", cb_n), ("e", cb_e)):
        g = NB * cb
        gp = int(math.ceil(g / XG)) * XG
        r = gp * P
        kinds.append(dict(
            tag=tg, cb=cb, g=g, gp=gp, r=r,
            x=nc.dram_tensor(f"x{tg}", [gp, P, 257], F16, kind="ExternalInput"),
            xt=nc.dram_tensor(f"xt{tg}", [2, P, r], F16, kind="ExternalInput"),
            sg=nc.dram_tensor(f"sg{tg}", [P, g], F32, kind="ExternalInput"),
            w1=nc.dram_tensor(f"w1{tg}", [2, P, P], F16, kind="ExternalInput"),
            w2=nc.dram_tensor(f"w2{tg}", [P, 64], F16, kind="ExternalInput"),
            w3=nc.dram_tensor(f"w3{tg}", [64, 1], F16, kind="ExternalInput"),
            b1=nc.dram_tensor(f"b1{tg}", [P, 1], F32, kind="ExternalInput"),
            b2=nc.dram_tensor(f"b2{tg}", [64, 1], F32, kind="ExternalInput"),
            out=nc.dram_tensor(f"out{tg}", [NB * SEGB, 256], F32,
                               kind="ExternalOutput"),
        ))
    iota_d = nc.dram_tensor("iota", [P, SEGB], F32, kind="ExternalInput")
    with tile.TileContext(nc) as tc, ExitStack() as ctx:
        _trace(ctx, tc, kinds, iota_d)
    return nc


_BUILD_CACHE = {}
_LAST_RESULTS = None  # BassKernelResults from the most recent kernel() call


def _get_program(cb_n, cb_e):
    key = (cb_n, cb_e)
    if key not in _BUILD_CACHE:
        _BUILD_CACHE[key] = _build(cb_n, cb_e)
    return _BUILD_CACHE[key]


def kernel(emb_nodes, emb_edges, node_index, edge_index,
           Wn1, bn1, Wn2, bn2, Wn3, bn3,
           We1, be1, We2, be2, We3, be3, num_graphs=None, **_unused):
    emb_nodes = np.asarray(emb_nodes, np.float32)
    emb_edges = np.asarray(emb_edges, np.float32)
    node_index = np.asarray(node_index)
    edge_index = np.asarray(edge_index)

    cb_n = _cb_for(node_index)
    cb_e = _cb_for(edge_index)
    nc = _get_program(cb_n, cb_e)

    xn, xtn, sgn = _shard(emb_nodes, node_index, cb_n)
    xe, xte, sge = _shard(emb_edges, edge_index, cb_e)

    iota = np.ascontiguousarray(
        np.broadcast_to(np.arange(SEGB, dtype=np.float32), (P, SEGB)))
    consts = dict(
        w1n=np.asarray(Wn1, np.float16).reshape(2, P, P),
        w2n=np.asarray(Wn2, np.float16),
        w3n=np.asarray(Wn3, np.float16),
        b1n=np.asarray(bn1, np.float32).reshape(P, 1),
        b2n=np.asarray(bn2, np.float32).reshape(64, 1),
        w1e=np.asarray(We1, np.float16).reshape(2, P, P),
        w2e=np.asarray(We2, np.float16),
        w3e=np.asarray(We3, np.float16),
        b1e=np.asarray(be1, np.float32).reshape(P, 1),
        b2e=np.asarray(be2, np.float32).reshape(64, 1),
        iota=iota,
    )
    in_maps = []
    for c in range(N_CORES):
        m = dict(consts)
        m["xn"], m["xtn"], m["sgn"] = xn[c], xtn[c], sgn[c]
        m["xe"], m["xte"], m["sge"] = xe[c], xte[c], sge[c]
        in_maps.append(m)

    global _LAST_RESULTS
    res = run_bass_kernel_spmd(nc, in_maps, core_ids=list(range(N_CORES)))
    _LAST_RESULTS = res
    out_n = np.concatenate([res.results[c]["outn"] for c in range(N_CORES)], axis=0)
    out_e = np.concatenate([res.results[c]["oute"] for c in range(N_CORES)], axis=0)
    return out_n.astype(np.float32), out_e.astype(np.float32)
